# revision 2
# baseline (speedup 1.0000x reference)
"""Attention kernel v2: f32r single-pass QK + matmul-folded max subtraction.

Per core (8 cores: batch b = c//2, query-half h = c%2): q [2048, 64],
k/v [4096, 64].

Pipeline:
  prep:   DMA q/k/v; PE-transpose q -> qTb blocks [65, 512] f32r (row 64
          reserved for M), k -> kTp [65, 4096] f32r (row 64 = -1); v_pack
          [128, 32, 65] bf16 (ones col 64).
  maxQK:  per q-tile t: 3 f32r matmuls [128q, {1536,1536,1024}k] -> PSUM,
          DVE reduce_max -> per-piece maxes, Pool combines -> M_all[:, t]
          (f32r); per tile a tiny DMA hops M into qTb[b] row 64.
  mainQK: per block b, 16 granule-pairs: K=65 f32r matmuls
          [128k, 512q] (scores - M built in), Act exp(scale=64) ->
          attn [128, 32, 512] bf16.
  PV:     per tile: 32 accum matmuls lhsT=attn chunk, rhs=[v|1] ->
          [128q, 65]; DVE reciprocal(Z); Act copy*scale -> out; DMA out.
"""
import sys

sys.path.insert(0, "/opt/trn_rl_repo")
import numpy as np

import concourse.bass as bass
import concourse.tile as tile
from concourse import bacc, mybir
from concourse.bass_utils import run_bass_kernel_spmd
from concourse.masks import make_identity

f32 = mybir.dt.float32
f32r = mybir.dt.float32r
bf16 = mybir.dt.bfloat16
Exp = mybir.ActivationFunctionType.Exp
Copy = mybir.ActivationFunctionType.Copy
AX = mybir.AxisListType.X

B, N, D = 4, 4096, 64
NCORES = 8
NQ = 2048
NK = 4096
QT = NQ // 128      # 16 q tiles
NB = NQ // 512      # 4 q blocks
CH = NK // 128      # 32 key chunks
SCALE = 64.0        # sqrt(N)
PIECES = [(0, 1024), (1024, 1024), (2048, 1024), (3072, 1024)]  # maxQK strip split

_cached = {}


def build_program():
    nc = bacc.Bacc("TRN2", target_bir_lowering=False, debug=False, num_devices=NCORES)
    q_d = nc.dram_tensor("q", [NQ, D], f32, kind="ExternalInput").ap()
    k_d = nc.dram_tensor("k", [NK, D], f32, kind="ExternalInput").ap()
    v_d = nc.dram_tensor("v", [NK, D], f32, kind="ExternalInput").ap()
    o_d = nc.dram_tensor("o", [NQ, D], f32, kind="ExternalOutput").ap()
    o3 = o_d.rearrange("(t p) d -> p t d", p=128)

    with tile.TileContext(nc) as tc:
        import contextlib

        ctx = contextlib.ExitStack()
        with ctx:
            const = ctx.enter_context(tc.tile_pool(name="const", bufs=1))
            big = ctx.enter_context(tc.tile_pool(name="big", bufs=1))
            attnp = ctx.enter_context(tc.tile_pool(name="attnp", bufs=4))

            ident = const.tile([128, 128], f32)
            make_identity(nc, ident[:])

            q_nat = big.tile([128, QT, D], f32)
            k_nat = big.tile([128, CH // 2, 2 * D], f32)   # r=2 packed
            v_nat = big.tile([128, CH // 2, 2 * D], f32)
            qTb = [
                big.tile([65, 256], f32r, name=f"qTb{i}") for i in range(2 * NB)
            ]
            kTp = big.tile([65, NK], f32r)
            v_pack = big.tile([128, CH, 65], bf16)
            Mparts = big.tile([128, 4 * QT], f32)
            Mtmp = big.tile([128, QT], f32)
            M_all = big.tile([128, QT], f32r)
            rZ = big.tile([128, QT], f32)
            out_sb = big.tile([128, QT, D], f32)

            # ---------------- input DMAs ----------------
            # k/v: r=2 row packing for 512B descriptor runs
            k4 = k_d.rearrange("(c p r) d -> p c (r d)", p=128, r=2)
            v4 = v_d.rearrange("(c p r) d -> p c (r d)", p=128, r=2)
            q3 = q_d.rearrange("(t p) d -> p t d", p=128)
            nc.sync.dma_start(out=k_nat[:, 0:4, :], in_=k4[:, 0:4, :])
            nc.sync.dma_start(out=q_nat[:, 0:4, :], in_=q3[:, 0:4, :])
            for g in range(1, 4):
                sl = slice(g * 4, (g + 1) * 4)
                nc.sync.dma_start(out=k_nat[:, sl, :], in_=k4[:, sl, :])
            nc.sync.dma_start(out=q_nat[:, 4:16, :], in_=q3[:, 4:16, :])
            for g in range(2):
                sl = slice(g * 8, (g + 1) * 8)
                nc.sync.dma_start(out=v_nat[:, sl, :], in_=v4[:, sl, :])

            nc.gpsimd.memset(kTp[64:65, :].bitcast(f32), -1.0)

            # v_pack: key slot j = 2c+r holds key 256c+2p+r (order matches kT)
            nc.gpsimd.tensor_copy(
                v_pack[:, :, 0:64],
                v_nat[:].rearrange("p c (r d) -> p (c r) d", r=2),
            )
            nc.gpsimd.memset(v_pack[:, :, 64:65], 1.0)

            # PE warmup: ramp the p-state during the input DMA wait
            warm = pmax_warm = None
            # ---------------- transposes (PE) + packs (Act) ----------------
            pmax = ctx.enter_context(tc.tile_pool(name="pmax", bufs=2, space="PSUM"))
            pmain = ctx.enter_context(tc.tile_pool(name="pmain", bufs=2, space="PSUM"))
            pstage = pmax



            def kprep(g):
                st = pstage.tile([64, 512], f32, tag="pm")
                for i in range(4):
                    j = g * 4 + i
                    c, r = j // 2, j % 2
                    nc.tensor.matmul(
                        st[:, i * 128 : (i + 1) * 128],
                        k_nat[:, c, 64 * r : 64 * r + 64],
                        ident[:],
                        is_transpose=True,
                    )
                nc.scalar.copy(kTp[0:64, g * 512 : (g + 1) * 512], st[:])

            def qprep(g):
                st = pstage.tile([64, 512], f32, tag="pm")
                for i in range(4):
                    t = g * 4 + i
                    nc.tensor.matmul(
                        st[:, i * 128 : (i + 1) * 128],
                        q_nat[:, t, :],
                        ident[:],
                        is_transpose=True,
                    )
                nc.scalar.copy(qTb[2 * g][0:64, :], st[:, 0:256])
                nc.scalar.copy(qTb[2 * g + 1][0:64, :], st[:, 256:512])

            def prepgen():
                # enough for maxgen(0) piece 0: k chunks 0-7 + qTb[0]
                kprep(0)
                kprep(1)
                qprep(0)
                yield
                for g in range(2, 8):
                    kprep(g)
                    yield
                for g in range(1, 4):
                    qprep(g)
                    yield

            attn_tiles = {}

            # ---------------- generators ----------------
            def maxgen(b):
                for jj in range(2):
                    t = 2 * b + jj
                    for pi, (o0, w) in enumerate(PIECES):
                        pm = pmax.tile([128, w], f32, tag="pm")
                        for s in range(w // 512):
                            nc.tensor.matmul(
                                pm[:, 512 * s : 512 * (s + 1)],
                                qTb[b][0:64, 128 * jj : 128 * (jj + 1)],
                                kTp[0:64, o0 + 512 * s : o0 + 512 * (s + 1)],
                                start=True,
                                stop=True,
                            )
                        nc.vector.reduce_max(
                            Mparts[:, 4 * t + pi : 4 * t + pi + 1], pm[:], axis=AX
                        )
                        yield
                    nc.vector.reduce_max(
                        M_all[:, t : t + 1], Mparts[:, 4 * t : 4 * t + 4], axis=AX
                    )
                    nc.sync.dma_start(
                        out=qTb[b][64:65, 128 * jj : 128 * (jj + 1)],
                        in_=M_all[:, t : t + 1],
                    )
                    yield

            def maingen(b):
                at = attnp.tile([128, CH, 256], bf16, tag="at")
                attn_tiles[b] = at
                for g in range(8):
                    ps = pmain.tile([128, 1024], f32, tag="ps")
                    for h in range(4):
                        c = 4 * g + h
                        nc.tensor.matmul(
                            ps[:, 256 * h : 256 * (h + 1)],
                            kTp[:, 128 * c : 128 * (c + 1)],
                            qTb[b][:],
                            start=True,
                            stop=True,
                        )
                    nc.scalar.activation(
                        out=at[:, 4 * g : 4 * g + 4, :],
                        in_=ps[:].rearrange("p (c q) -> p c q", c=4),
                        func=Exp,
                        bias=0.0,
                        scale=SCALE,
                    )
                    yield

            def pvgen(b):
                at = attn_tiles[b]
                for jj in range(2):
                    t = 2 * b + jj
                    pv = pmain.tile([128, 65], f32, tag="ps")
                    for c in range(CH):
                        nc.tensor.matmul(
                            pv[:],
                            at[:, c, 128 * jj : 128 * (jj + 1)],
                            v_pack[:, c, :],
                            start=(c == 0),
                            stop=(c == CH - 1),
                        )
                        if c % 8 == 7 and c != CH - 1:
                            yield
                    nc.vector.reciprocal(rZ[:, t : t + 1], pv[:, 64:65])
                    nc.scalar.activation(
                        out=out_sb[:, t, :],
                        in_=pv[:, 0:64],
                        func=Copy,
                        bias=0.0,
                        scale=rZ[:, t : t + 1],
                    )
                    yield
                    nc.sync.dma_start(
                        out=o3[:, t : t + 1, :], in_=out_sb[:, t : t + 1, :]
                    )
                yield

            def run_interleaved(gens):
                alive = [[g, r] for g, r in gens]
                while alive:
                    for item in list(alive):
                        g, ratio = item
                        for _ in range(ratio):
                            try:
                                next(g)
                            except StopIteration:
                                alive.remove(item)
                                break

            # continuous weave: max(g) | main(g-1) | pv(g-2), no phase barriers
            def chain(genf, lo, hi):
                for g in range(lo, hi):
                    yield from genf(g)

            def step(g):
                try:
                    next(g)
                    return True
                except StopIteration:
                    return False

            pg = prepgen()
            next(pg)
            mx = chain(maxgen, 0, 8)
            mn = chain(maingen, 0, 8)
            pv = chain(pvgen, 0, 8)
            # prep phase: 2 prep-steps per max-piece
            alive_pg = True
            for i in range(10):
                if alive_pg:
                    alive_pg = step(pg) and step(pg)
                step(mx)
            # steady state: per group-cycle: 10 max, 8 main, 8 pv interleaved
            for gc in range(1, 11):
                for i in range(10):
                    if gc < 8:
                        step(mx)
                    if i < 8:
                        step(mn)
                        if gc >= 2:
                            step(pv)
            while step(mx):
                pass
            while step(mn):
                pass
            while step(pv):
                pass

    nc.compile()
    return nc


def kernel(q, k, v):
    if "nc" not in _cached:
        _cached["nc"] = build_program()
    nc = _cached["nc"]
    in_maps = []
    for c in range(NCORES):
        b, h = c // 2, c % 2
        in_maps.append(
            {
                "q": np.ascontiguousarray(q[b, h * NQ : (h + 1) * NQ, :]),
                "k": np.ascontiguousarray(k[b]),
                "v": np.ascontiguousarray(v[b]),
            }
        )
    res = run_bass_kernel_spmd(nc, in_maps, list(range(NCORES)))
    out = np.empty((B, N, D), dtype=np.float32)
    for c in range(NCORES):
        b, h = c // 2, c % 2
        out[b, h * NQ : (h + 1) * NQ, :] = res.results[c]["o"]
    return out


# revision 3
# speedup vs baseline: 1.0003x; 1.0003x over previous
"""Attention kernel v2: f32r single-pass QK + matmul-folded max subtraction.

Per core (8 cores: batch b = c//2, query-half h = c%2): q [2048, 64],
k/v [4096, 64].

Pipeline:
  prep:   DMA q/k/v; PE-transpose q -> qTb blocks [65, 512] f32r (row 64
          reserved for M), k -> kTp [65, 4096] f32r (row 64 = -1); v_pack
          [128, 32, 65] bf16 (ones col 64).
  maxQK:  per q-tile t: 3 f32r matmuls [128q, {1536,1536,1024}k] -> PSUM,
          DVE reduce_max -> per-piece maxes, Pool combines -> M_all[:, t]
          (f32r); per tile a tiny DMA hops M into qTb[b] row 64.
  mainQK: per block b, 16 granule-pairs: K=65 f32r matmuls
          [128k, 512q] (scores - M built in), Act exp(scale=64) ->
          attn [128, 32, 512] bf16.
  PV:     per tile: 32 accum matmuls lhsT=attn chunk, rhs=[v|1] ->
          [128q, 65]; DVE reciprocal(Z); Act copy*scale -> out; DMA out.
"""
import sys

sys.path.insert(0, "/opt/trn_rl_repo")
import numpy as np

import concourse.bass as bass
import concourse.tile as tile
from concourse import bacc, mybir
from concourse.bass_utils import run_bass_kernel_spmd
from concourse.masks import make_identity

f32 = mybir.dt.float32
f32r = mybir.dt.float32r
bf16 = mybir.dt.bfloat16
Exp = mybir.ActivationFunctionType.Exp
Copy = mybir.ActivationFunctionType.Copy
AX = mybir.AxisListType.X

B, N, D = 4, 4096, 64
NCORES = 8
NQ = 2048
NK = 4096
QT = NQ // 128      # 16 q tiles
NB = NQ // 512      # 4 q blocks
CH = NK // 128      # 32 key chunks
SCALE = 64.0        # sqrt(N)
PIECES = [(0, 1024), (1024, 1024), (2048, 1024), (3072, 1024)]  # maxQK strip split

_cached = {}


def build_program():
    nc = bacc.Bacc("TRN2", target_bir_lowering=False, debug=False, num_devices=NCORES)
    q_d = nc.dram_tensor("q", [NQ, D], f32, kind="ExternalInput").ap()
    k_d = nc.dram_tensor("k", [NK, D], f32, kind="ExternalInput").ap()
    v_d = nc.dram_tensor("v", [NK, D], f32, kind="ExternalInput").ap()
    o_d = nc.dram_tensor("o", [NQ, D], f32, kind="ExternalOutput").ap()
    o3 = o_d.rearrange("(t p) d -> p t d", p=128)

    with tile.TileContext(nc) as tc:
        import contextlib

        ctx = contextlib.ExitStack()
        with ctx:
            const = ctx.enter_context(tc.tile_pool(name="const", bufs=1))
            big = ctx.enter_context(tc.tile_pool(name="big", bufs=1))
            attnp = ctx.enter_context(tc.tile_pool(name="attnp", bufs=4))

            ident = const.tile([128, 128], f32)
            make_identity(nc, ident[:])

            q_nat = big.tile([128, QT, D], f32)
            k_nat = big.tile([128, CH // 2, 2 * D], f32)   # r=2 packed
            v_nat = big.tile([128, CH // 2, 2 * D], f32)
            qTb = [
                big.tile([65, 256], f32r, name=f"qTb{i}") for i in range(2 * NB)
            ]
            kTp = big.tile([65, NK], f32r)
            v_pack = big.tile([128, CH, 65], bf16)
            Mparts = big.tile([128, 4 * QT], f32)
            Mtmp = big.tile([128, QT], f32)
            M_all = big.tile([128, QT], f32r)
            rZ = big.tile([128, QT], f32)
            out_sb = big.tile([128, QT, D], f32)

            # ---------------- input DMAs ----------------
            # k/v: r=2 row packing for 512B descriptor runs
            k4 = k_d.rearrange("(c p r) d -> p c (r d)", p=128, r=2)
            v4 = v_d.rearrange("(c p r) d -> p c (r d)", p=128, r=2)
            q3 = q_d.rearrange("(t p) d -> p t d", p=128)
            nc.sync.dma_start(out=k_nat[:, 0:2, :], in_=k4[:, 0:2, :])
            nc.sync.dma_start(out=q_nat[:, 0:4, :], in_=q3[:, 0:4, :])
            nc.sync.dma_start(out=k_nat[:, 2:4, :], in_=k4[:, 2:4, :])
            for g in range(1, 4):
                sl = slice(g * 4, (g + 1) * 4)
                nc.sync.dma_start(out=k_nat[:, sl, :], in_=k4[:, sl, :])
            nc.sync.dma_start(out=q_nat[:, 4:16, :], in_=q3[:, 4:16, :])
            for g in range(2):
                sl = slice(g * 8, (g + 1) * 8)
                nc.sync.dma_start(out=v_nat[:, sl, :], in_=v4[:, sl, :])

            nc.gpsimd.memset(kTp[64:65, :].bitcast(f32), -1.0)

            # v_pack: key slot j = 2c+r holds key 256c+2p+r (order matches kT)
            nc.gpsimd.tensor_copy(
                v_pack[:, :, 0:64],
                v_nat[:].rearrange("p c (r d) -> p (c r) d", r=2),
            )
            nc.gpsimd.memset(v_pack[:, :, 64:65], 1.0)

            # PE warmup: ramp the p-state during the input DMA wait
            warm = pmax_warm = None
            # ---------------- transposes (PE) + packs (Act) ----------------
            pmax = ctx.enter_context(tc.tile_pool(name="pmax", bufs=2, space="PSUM"))
            pmain = ctx.enter_context(tc.tile_pool(name="pmain", bufs=2, space="PSUM"))
            pstage = pmax



            def kprep(g):
                st = pstage.tile([64, 512], f32, tag="pm")
                for i in range(4):
                    j = g * 4 + i
                    c, r = j // 2, j % 2
                    nc.tensor.matmul(
                        st[:, i * 128 : (i + 1) * 128],
                        k_nat[:, c, 64 * r : 64 * r + 64],
                        ident[:],
                        is_transpose=True,
                    )
                nc.scalar.copy(kTp[0:64, g * 512 : (g + 1) * 512], st[:])

            def qprep(g):
                st = pstage.tile([64, 512], f32, tag="pm")
                for i in range(4):
                    t = g * 4 + i
                    nc.tensor.matmul(
                        st[:, i * 128 : (i + 1) * 128],
                        q_nat[:, t, :],
                        ident[:],
                        is_transpose=True,
                    )
                nc.scalar.copy(qTb[2 * g][0:64, :], st[:, 0:256])
                nc.scalar.copy(qTb[2 * g + 1][0:64, :], st[:, 256:512])

            def prepgen():
                # enough for maxgen(0) piece 0: k chunks 0-7 + qTb[0]
                kprep(0)
                kprep(1)
                qprep(0)
                yield
                for g in range(2, 8):
                    kprep(g)
                    yield
                for g in range(1, 4):
                    qprep(g)
                    yield

            attn_tiles = {}

            # ---------------- generators ----------------
            def maxgen(b):
                for jj in range(2):
                    t = 2 * b + jj
                    for pi, (o0, w) in enumerate(PIECES):
                        pm = pmax.tile([128, w], f32, tag="pm")
                        for s in range(w // 512):
                            nc.tensor.matmul(
                                pm[:, 512 * s : 512 * (s + 1)],
                                qTb[b][0:64, 128 * jj : 128 * (jj + 1)],
                                kTp[0:64, o0 + 512 * s : o0 + 512 * (s + 1)],
                                start=True,
                                stop=True,
                            )
                        nc.vector.reduce_max(
                            Mparts[:, 4 * t + pi : 4 * t + pi + 1], pm[:], axis=AX
                        )
                        yield
                    nc.vector.reduce_max(
                        M_all[:, t : t + 1], Mparts[:, 4 * t : 4 * t + 4], axis=AX
                    )
                    nc.sync.dma_start(
                        out=qTb[b][64:65, 128 * jj : 128 * (jj + 1)],
                        in_=M_all[:, t : t + 1],
                    )
                    yield

            def maingen(b):
                at = attnp.tile([128, CH, 256], bf16, tag="at")
                attn_tiles[b] = at
                for g in range(8):
                    ps = pmain.tile([128, 1024], f32, tag="ps")
                    for h in range(4):
                        c = 4 * g + h
                        nc.tensor.matmul(
                            ps[:, 256 * h : 256 * (h + 1)],
                            kTp[:, 128 * c : 128 * (c + 1)],
                            qTb[b][:],
                            start=True,
                            stop=True,
                        )
                    nc.scalar.activation(
                        out=at[:, 4 * g : 4 * g + 4, :],
                        in_=ps[:].rearrange("p (c q) -> p c q", c=4),
                        func=Exp,
                        bias=0.0,
                        scale=SCALE,
                    )
                    yield

            def pvgen(b):
                at = attn_tiles[b]
                for jj in range(2):
                    t = 2 * b + jj
                    pv = pmain.tile([128, 65], f32, tag="ps")
                    for c in range(CH):
                        nc.tensor.matmul(
                            pv[:],
                            at[:, c, 128 * jj : 128 * (jj + 1)],
                            v_pack[:, c, :],
                            start=(c == 0),
                            stop=(c == CH - 1),
                        )
                        if c % 8 == 7 and c != CH - 1:
                            yield
                    nc.vector.reciprocal(rZ[:, t : t + 1], pv[:, 64:65])
                    nc.scalar.activation(
                        out=out_sb[:, t, :],
                        in_=pv[:, 0:64],
                        func=Copy,
                        bias=0.0,
                        scale=rZ[:, t : t + 1],
                    )
                    yield
                    nc.sync.dma_start(
                        out=o3[:, t : t + 1, :], in_=out_sb[:, t : t + 1, :]
                    )
                yield

            def run_interleaved(gens):
                alive = [[g, r] for g, r in gens]
                while alive:
                    for item in list(alive):
                        g, ratio = item
                        for _ in range(ratio):
                            try:
                                next(g)
                            except StopIteration:
                                alive.remove(item)
                                break

            # continuous weave: max(g) | main(g-1) | pv(g-2), no phase barriers
            def chain(genf, lo, hi):
                for g in range(lo, hi):
                    yield from genf(g)

            def step(g):
                try:
                    next(g)
                    return True
                except StopIteration:
                    return False

            pg = prepgen()
            next(pg)
            mx = chain(maxgen, 0, 8)
            mn = chain(maingen, 0, 8)
            pv = chain(pvgen, 0, 8)
            # prep phase: 2 prep-steps per max-piece
            alive_pg = True
            for i in range(10):
                if alive_pg:
                    alive_pg = step(pg) and step(pg)
                step(mx)
            # steady state: per group-cycle: 10 max, 8 main, 8 pv interleaved
            for gc in range(1, 11):
                for i in range(10):
                    if gc < 8:
                        step(mx)
                    if i < 8:
                        step(mn)
                        if gc >= 2:
                            step(pv)
            while step(mx):
                pass
            while step(mn):
                pass
            while step(pv):
                pass

    nc.compile()
    return nc


def kernel(q, k, v):
    if "nc" not in _cached:
        _cached["nc"] = build_program()
    nc = _cached["nc"]
    in_maps = []
    for c in range(NCORES):
        b, h = c // 2, c % 2
        in_maps.append(
            {
                "q": np.ascontiguousarray(q[b, h * NQ : (h + 1) * NQ, :]),
                "k": np.ascontiguousarray(k[b]),
                "v": np.ascontiguousarray(v[b]),
            }
        )
    res = run_bass_kernel_spmd(nc, in_maps, list(range(NCORES)))
    out = np.empty((B, N, D), dtype=np.float32)
    for c in range(NCORES):
        b, h = c // 2, c % 2
        out[b, h * NQ : (h + 1) * NQ, :] = res.results[c]["o"]
    return out
